# revision 5
# baseline (speedup 1.0000x reference)
"""Trainium2 Bass kernel for nn_Cross_Attn (sparse_attention).

Reference computation (B=4, C=384, N=2048, K=16, G=32):
  q  = Wq@feat + bq                            [B,N,C]
  gk = Wk@grouped_feat + bk                    [B,N,C,K]
  s  = (q . gk) * C^-0.5                       [B,N,K]
  p  = softmax_k(mask(s, count))               [B,N,K]   (rows of attn identical)
  v  = relu(GroupNorm_G(Wv@grouped_feat + bv)) [B,C,N,K]
  out[b,c,n] = K * sum_k p[b,n,k] * v[b,c,n,k]

Algebraic restructure used here:
  * attn is rank-1 over the query axis -> out = K * sum_k p * v.
  * s = (Wk^T q) . g + q.bk; the q.bk term is constant over k and softmax
    drops it, so s = u . g with u = (scale Wk^T Wq) feat + scale Wk^T bq.
  * GroupNorm statistics couple all of N, so the kernel runs two SPMD
    launches over N-shards: A computes p + per-channel mean/var (bn_stats
    over v0 = Wv@g); the host merges stats into per-(b,c) affine scale/bias;
    B recomputes v' = (alpha*Wv)@g + bias' and accumulates
    out = sum_k relu(v')*p  (relu(z)*p == relu(z*p) since p>=0).

Sharding: N axis split contiguously across the 8 cores (256 points each);
all weights replicated. Only host-merged 6KB of statistics cross cores.
"""

import numpy as np

import concourse.bass as bass
import concourse.mybir as mybir
import concourse.tile as tile
from concourse import bass_utils

B, C, N, K, G = 4, 384, 2048, 16, 32
EPS = 1e-5
NCORES = 8
NS = N // NCORES          # n-points per core
CT = C // 128             # 128-partition channel tiles
NHALF = NS // 128         # 128-n' scores tiles per (b, core)
NK = NS * K               # free elems per (b, core)
HNK = 128 * K             # free elems per (b, half)
CH = 512                  # matmul moving chunk (fp32 max, 1 PSUM bank)
SCALE = float(C) ** -0.5

F32 = mybir.dt.float32
F32R = mybir.dt.float32r
BF16 = mybir.dt.bfloat16

# --- tunables -------------------------------------------------------------
MM_F32R = True            # run the big GEMMs as float32r (1 cyc/row vs 4)
DT_MM = F32R if MM_F32R else F32   # dtype of matmul operand tensors/tiles
_wait_counter = [0]


def _fix_excess_waits(nc, max_waits=1):
    """Split instructions carrying more sync waits than this walrus accepts
    (TileContext's tail drain waits on the whole global clock)."""
    for f in nc.m.functions:
        for bb in f.blocks:
            out = []
            for ins in bb.instructions:
                si = ins.sync_info
                if si is not None and si.on_wait and len(si.on_wait) > max_waits:
                    waits = list(si.on_wait)
                    head, tail = waits[:-max_waits], waits[-max_waits:]
                    for i in range(0, len(head), max_waits):
                        _wait_counter[0] += 1
                        nop = mybir.InstNoOp(
                            name=f"I-waitsplit-{_wait_counter[0]}", ins=[], outs=[]
                        )
                        nop.engine = ins.engine
                        nop.sync_info = type(si)(
                            on_wait=head[i : i + max_waits], on_update=[]
                        )
                        out.append(nop)
                    ins.sync_info = type(si)(
                        on_wait=tail, on_update=list(si.on_update or [])
                    )
                out.append(ins)
            bb.instructions[:] = out
    return nc




def build_a(fix=True):
    """Launch A: scores+softmax -> p;  bn stats of v0 = Wv@g (no bias)."""
    nc = bass.Bass("TRN2", target_bir_lowering=False, debug=False)
    g_d = nc.dram_tensor("g", [B, C, NS, K], DT_MM, kind="ExternalInput")
    feat_d = nc.dram_tensor("feat", [B, C, NS], DT_MM, kind="ExternalInput")
    count_d = nc.dram_tensor("count", [B, NS], mybir.dt.int32, kind="ExternalInput")
    mt_d = nc.dram_tensor("Mt", [C, C], DT_MM, kind="ExternalInput")
    cvec_d = nc.dram_tensor("cvec", [C], F32, kind="ExternalInput")
    wvt_d = nc.dram_tensor("WvT", [C, C], DT_MM, kind="ExternalInput")
    iota_d = nc.dram_tensor("iota", [128, K], F32, kind="ExternalInput")
    diag_d = nc.dram_tensor("D", [128, 128 * K], F32, kind="ExternalInput")
    p_d = nc.dram_tensor("p", [B, NS, K], F32, kind="ExternalOutput")
    stats_d = nc.dram_tensor("stats", [128, CT, B, 2], F32, kind="ExternalOutput")

    NCHUNK = HNK // CH    # 512-col chunks per (b, half)
    NPC = CH // K         # n' values covered per chunk (32)

    with tile.TileContext(nc) as tc:
        with (
            tc.tile_pool(name="consts", bufs=1) as consts,
            tc.tile_pool(name="gpool", bufs=3) as gpool,
            tc.tile_pool(name="work", bufs=3) as work,
            tc.tile_pool(name="small", bufs=4) as small,
            tc.tile_pool(name="acc", bufs=1) as accp,
            tc.tile_pool(name="ps_u", bufs=2, space="PSUM") as ps_u,
            tc.tile_pool(name="ps_s", bufs=2, space="PSUM") as ps_s,
            tc.tile_pool(name="ps_v", bufs=3, space="PSUM") as ps_v,
        ):
            mt_sb = consts.tile([128, CT, C], DT_MM)
            nc.sync.dma_start(
                mt_sb[:], mt_d[:].rearrange("(t p) c -> p t c", p=128)
            )
            wvt_sb = consts.tile([128, CT, C], DT_MM)
            nc.sync.dma_start(
                wvt_sb[:], wvt_d[:].rearrange("(t p) c -> p t c", p=128)
            )
            cvec_sb = consts.tile([128, CT], F32)
            nc.sync.dma_start(cvec_sb[:], cvec_d[:].rearrange("(t p) -> p t", p=128))
            iota_sb = consts.tile([128, K], F32)
            nc.sync.dma_start(iota_sb[:], iota_d[:])
            diag_sb = consts.tile([128, 128 * K], F32)
            nc.sync.dma_start(diag_sb[:], diag_d[:])

            # ---- u[b] = Mt^T@feat + cvec, laid out [cu-part, ct, b, n] ----
            u_sb = accp.tile([128, CT, B, NS], DT_MM)
            for b in range(B):
                feat_t = work.tile([128, CT, NS], DT_MM, tag="feat")
                nc.sync.dma_start(
                    feat_t[:], feat_d[b].rearrange("(t p) n -> p t n", p=128)
                )
                for cu in range(CT):
                    ups = ps_u.tile([128, NS], F32)
                    for cq in range(CT):
                        nc.tensor.matmul(
                            ups[:],
                            (mt_sb[:, cq, cu * 128 : (cu + 1) * 128]),
                            (feat_t[:, cq, :]),
                            start=(cq == 0),
                            stop=(cq == CT - 1),
                        )
                    nc.scalar.activation(
                        u_sb[:, cu, b, :],
                        ups[:],
                        mybir.ActivationFunctionType.Identity,
                        bias=cvec_sb[:, cu : cu + 1],
                        scale=1.0,
                    )

            # ---- main pass over g ----
            bnrec = accp.tile([128, CT, B, NHALF * NCHUNK, 6], F32)
            for b in range(B):
                for h in range(NHALF):
                    g_sb = gpool.tile([128, CT, HNK], DT_MM, tag="g")
                    for ct in range(CT):
                        nc.sync.dma_start(
                            g_sb[:, ct, :],
                            g_d[b]
                            .rearrange("(t p) n k -> p t (n k)", p=128)[
                                :, ct, h * HNK : (h + 1) * HNK
                            ],
                        )

                    # scores: all-pairs matmul, diagonal extraction
                    sslot = small.tile([128, NCHUNK, K], F32, tag="sslot")
                    for ci in range(NCHUNK):
                        aps = ps_s.tile([128, CH], F32)
                        for ct in range(CT):
                            nc.tensor.matmul(
                                aps[:],
                                (u_sb[:, ct, b, h * 128 : (h + 1) * 128]),
                                (g_sb[:, ct, ci * CH : (ci + 1) * CH]),
                                start=(ct == 0),
                                stop=(ct == CT - 1),
                            )
                        td = work.tile([128, CH], F32, tag="td")
                        nc.vector.tensor_tensor(
                            td[:],
                            aps[:],
                            diag_sb[:, ci * CH : (ci + 1) * CH],
                            op=mybir.AluOpType.mult,
                        )
                        # sum over the 32 n-columns (k kept): view [p, k, n]
                        nc.vector.tensor_reduce(
                            sslot[:, ci, :],
                            td[:].rearrange("p (n k) -> p k n", k=K),
                            axis=mybir.AxisListType.X,
                            op=mybir.AluOpType.add,
                        )
                    s_sb = small.tile([128, K], F32, tag="s")
                    nc.vector.tensor_reduce(
                        s_sb[:],
                        sslot[:].rearrange("p c k -> p k c"),
                        axis=mybir.AxisListType.X,
                        op=mybir.AluOpType.add,
                    )

                    # masked softmax over k (x16 folded in)
                    cnt_i = small.tile([128, 1], mybir.dt.int32, tag="cnti")
                    nc.sync.dma_start(
                        cnt_i[:], count_d[b, h * 128 : (h + 1) * 128].unsqueeze(-1)
                    )
                    cnt_f = small.tile([128, 1], F32, tag="cntf")
                    nc.vector.tensor_copy(cnt_f[:], cnt_i[:])
                    nc.vector.tensor_scalar_max(cnt_f[:], cnt_f[:], 1.0)
                    m_sb = small.tile([128, K], F32, tag="m")
                    nc.vector.tensor_tensor(
                        m_sb[:],
                        iota_sb[:],
                        cnt_f[:].broadcast_to((128, K)),
                        op=mybir.AluOpType.is_lt,
                    )
                    mx = small.tile([128, 1], F32, tag="mx")
                    nc.vector.tensor_reduce(
                        mx[:], s_sb[:], axis=mybir.AxisListType.X,
                        op=mybir.AluOpType.max,
                    )
                    negmx = small.tile([128, 1], F32, tag="negmx")
                    nc.vector.tensor_scalar_mul(negmx[:], mx[:], -1.0)
                    e_sb = small.tile([128, K], F32, tag="e")
                    nc.scalar.activation(
                        e_sb[:],
                        s_sb[:],
                        mybir.ActivationFunctionType.Exp,
                        bias=negmx[:, 0:1],
                        scale=1.0,
                    )
                    em = small.tile([128, K], F32, tag="em")
                    nc.vector.tensor_tensor(
                        em[:], e_sb[:], m_sb[:], op=mybir.AluOpType.mult
                    )
                    sm = small.tile([128, 1], F32, tag="sm")
                    nc.vector.tensor_reduce(
                        sm[:], em[:], axis=mybir.AxisListType.X,
                        op=mybir.AluOpType.add,
                    )
                    rec = small.tile([128, 1], F32, tag="rec")
                    nc.vector.reciprocal(rec[:], sm[:])
                    nc.vector.tensor_scalar_mul(rec[:], rec[:], float(K))
                    p_t = small.tile([128, K], F32, tag="pt")
                    nc.vector.tensor_scalar_mul(p_t[:], em[:], rec[:, 0:1])
                    nc.sync.dma_start(
                        p_d[b, h * 128 : (h + 1) * 128, :], p_t[:]
                    )

                    # v0 = Wv@g (no bias) -> bn_stats per 512-chunk
                    for co in range(CT):
                        for ci in range(NCHUNK):
                            vps = ps_v.tile([128, CH], F32)
                            for cin in range(CT):
                                nc.tensor.matmul(
                                    vps[:],
                                    (wvt_sb[:, cin, co * 128 : (co + 1) * 128]),
                                    (g_sb[:, cin, ci * CH : (ci + 1) * CH]),
                                    start=(cin == 0),
                                    stop=(cin == CT - 1),
                                )
                            nc.vector.bn_stats(
                                bnrec[:, co, b, h * NCHUNK + ci, :], vps[:]
                            )

            stats_sb = accp.tile([128, CT, B, 2], F32)
            for co in range(CT):
                for b in range(B):
                    nc.vector.bn_aggr(stats_sb[:, co, b, :], bnrec[:, co, b, :, :])
            nc.sync.dma_start(stats_d[:], stats_sb[:])

    return _fix_excess_waits(nc) if fix else nc


def build_b(fix=True):
    """Launch B: out[c,n] = sum_k relu((alpha*Wv)@g + bias')*p ."""
    nc = bass.Bass("TRN2", target_bir_lowering=False, debug=False)
    g_d = nc.dram_tensor("g", [B, C, NS, K], DT_MM, kind="ExternalInput")
    p_d = nc.dram_tensor("p", [B, NS, K], F32, kind="ExternalInput")
    wt_d = nc.dram_tensor("WtT", [B, C, C], DT_MM, kind="ExternalInput")
    b2_d = nc.dram_tensor("bias2", [B, C], DT_MM, kind="ExternalInput")
    ones_d = nc.dram_tensor("ones", [1, CH], DT_MM, kind="ExternalInput")
    out_d = nc.dram_tensor("out", [B, C, NS], F32, kind="ExternalOutput")

    NCHUNK = HNK // CH
    NPC = CH // K

    with tile.TileContext(nc) as tc:
        with (
            tc.tile_pool(name="consts", bufs=1) as consts,
            tc.tile_pool(name="gpool", bufs=3) as gpool,
            tc.tile_pool(name="work", bufs=3) as work,
            tc.tile_pool(name="prep", bufs=2) as prep,
            tc.tile_pool(name="acc", bufs=1) as accp,
            tc.tile_pool(name="ps_v", bufs=4, space="PSUM") as ps_v,
        ):
            wt_sb = consts.tile([128, B, CT, C], DT_MM)
            nc.sync.dma_start(
                wt_sb[:], wt_d[:].rearrange("b (t p) c -> p b t c", p=128)
            )
            b2_sb = consts.tile([1, B, C], DT_MM)
            nc.sync.dma_start(b2_sb[:], b2_d[:].rearrange("b c -> () b c"))
            ones_sb = consts.tile([1, CH], DT_MM)
            nc.sync.dma_start(ones_sb[:], ones_d[:])

            out_acc = accp.tile([128, CT, B, NS], F32)
            for b in range(B):
                # p replicated across partitions: [128, NK]
                p_rep = prep.tile([128, NK], F32, tag="prep")
                nc.sync.dma_start(
                    p_rep[:],
                    p_d[b].rearrange("n k -> (n k)").unsqueeze(0)
                    .partition_broadcast(128)[:, 0, :],
                )
                for h in range(NHALF):
                    g_sb = gpool.tile([128, CT, HNK], DT_MM, tag="g")
                    for ct in range(CT):
                        nc.sync.dma_start(
                            g_sb[:, ct, :],
                            g_d[b]
                            .rearrange("(t p) n k -> p t (n k)", p=128)[
                                :, ct, h * HNK : (h + 1) * HNK
                            ],
                        )
                    for co in range(CT):
                        for ci in range(NCHUNK):
                            vps = ps_v.tile([128, CH], F32)
                            for cin in range(CT):
                                nc.tensor.matmul(
                                    vps[:],
                                    (wt_sb[:, b, cin, co * 128 : (co + 1) * 128]),
                                    (g_sb[:, cin, ci * CH : (ci + 1) * CH]),
                                    start=(cin == 0),
                                    stop=False,
                                )
                            nc.tensor.matmul(
                                vps[:],
                                (b2_sb[:, b, co * 128 : (co + 1) * 128]),
                                (ones_sb[:]),
                                start=False,
                                stop=True,
                            )
                            # t = relu(v') * p ; then sum over k
                            t_sb = work.tile([128, CH], F32, tag="t")
                            nc.vector.scalar_tensor_tensor(
                                t_sb[:],
                                vps[:],
                                0.0,
                                p_rep[:, h * HNK + ci * CH : h * HNK + (ci + 1) * CH],
                                op0=mybir.AluOpType.max,
                                op1=mybir.AluOpType.mult,
                            )
                            nc.vector.tensor_reduce(
                                out_acc[
                                    :, co, b,
                                    h * 128 + ci * NPC : h * 128 + (ci + 1) * NPC,
                                ],
                                t_sb[:].rearrange("p (n k) -> p n k", k=K),
                                axis=mybir.AxisListType.X,
                                op=mybir.AluOpType.add,
                            )
            for co in range(CT):
                for b in range(B):
                    nc.sync.dma_start(
                        out_d[b, co * 128 : (co + 1) * 128, :], out_acc[:, co, b, :]
                    )

    return _fix_excess_waits(nc) if fix else nc


# ---------------------------------------------------------------------------
_built = {}


def _get_modules():
    if "a" not in _built:
        _built["a"] = build_a()
        _built["b"] = build_b()
    return _built["a"], _built["b"]


def host_prep(Wq, bq, Wk, bk):
    Mt = (SCALE * (Wq.T.astype(np.float64) @ Wk.astype(np.float64))).astype(np.float32)
    cvec = (SCALE * (Wk.T.astype(np.float64) @ bq.astype(np.float64))).astype(
        np.float32
    )
    iota = np.broadcast_to(np.arange(K, dtype=np.float32), (128, K)).copy()
    # D[p, (n,k)] = 1 where the all-pairs column's n (within a 512-col chunk
    # window) matches partition p: chunk ci covers n in [ci*32, ci*32+32).
    pidx = np.arange(128)
    nidx = np.arange(128 * K) // K  # column -> n index within the half
    D = (pidx[:, None] == nidx[None, :]).astype(np.float32)
    return Mt, cvec, iota, D


def host_stats_to_affine(stats_all, bv, gn_w, gn_b):
    """stats_all: [NCORES, 128, CT, B, 2] -> scale/bias [B, C] fp32 pair."""
    st = stats_all.astype(np.float64)
    # [core, p, ct, b, {mean, var}] -> per-channel c = ct*128 + p
    mean0 = st[..., 0].transpose(2, 1, 0, 3).reshape(C, NCORES, B)
    var0 = st[..., 1].transpose(2, 1, 0, 3).reshape(C, NCORES, B)
    # merge cores (equal counts): E[x], E[x^2]
    m_c = mean0.mean(axis=1) + bv.astype(np.float64)[:, None]        # [C, B]
    e2_c = (var0 + mean0**2).mean(axis=1) + (
        2 * mean0.mean(axis=1) * bv.astype(np.float64)[:, None]
        + (bv.astype(np.float64) ** 2)[:, None]
    )
    # group over channels
    m_g = m_c.reshape(G, C // G, B).mean(axis=1)                     # [G, B]
    e2_g = e2_c.reshape(G, C // G, B).mean(axis=1)
    var_g = e2_g - m_g**2
    rstd = 1.0 / np.sqrt(var_g + EPS)                                # [G, B]
    rstd_c = np.repeat(rstd, C // G, axis=0)                         # [C, B]
    mu_c = np.repeat(m_g, C // G, axis=0)
    alpha = gn_w.astype(np.float64)[:, None] * rstd_c                # [C, B]
    beta = gn_b.astype(np.float64)[:, None] - mu_c * alpha
    scaleB = alpha.T.astype(np.float32)                              # [B, C]
    biasB = (alpha * bv.astype(np.float64)[:, None] + beta).T.astype(np.float32)
    return scaleB, biasB


def kernel(feat, grouped_feat, count, Wq, bq, Wk, bk, Wv, bv, gn_w, gn_b):
    feat = np.asarray(feat, dtype=np.float32)
    g = np.asarray(grouped_feat, dtype=np.float32)
    count = np.asarray(count, dtype=np.int32)
    Wq, bq, Wk, bk, Wv, bv, gn_w, gn_b = (
        np.asarray(a, dtype=np.float32) for a in (Wq, bq, Wk, bk, Wv, bv, gn_w, gn_b)
    )
    nc_a, nc_b = _get_modules()
    Mt, cvec, iota, D = host_prep(Wq, bq, Wk, bk)
    WvT = np.ascontiguousarray(Wv.T)

    core_sl = [slice(i * NS, (i + 1) * NS) for i in range(NCORES)]
    in_a = [
        {
            "g": g[:, :, sl, :],
            "feat": feat[:, :, sl],
            "count": count[:, sl],
            "Mt": Mt,
            "cvec": cvec,
            "WvT": WvT,
            "iota": iota,
            "D": D,
        }
        for sl in core_sl
    ]
    res_a = bass_utils.run_bass_kernel_spmd(nc_a, in_a, core_ids=list(range(NCORES)))
    stats_all = np.stack([res_a.results[i]["stats"] for i in range(NCORES)])
    p_all = [res_a.results[i]["p"] for i in range(NCORES)]

    scaleB, biasB = host_stats_to_affine(stats_all, bv, gn_w, gn_b)
    WtT = np.ascontiguousarray(Wv.T[None, :, :] * scaleB[:, None, :])  # [B, cin, cout]

    in_b = [
        {
            "g": g[:, :, core_sl[i], :],
            "p": p_all[i],
            "WtT": WtT,
            "bias2": biasB,
            "ones": np.ones((1, CH), np.float32),
        }
        for i in range(NCORES)
    ]
    res_b = bass_utils.run_bass_kernel_spmd(nc_b, in_b, core_ids=list(range(NCORES)))
    out = np.concatenate(
        [res_b.results[i]["out"] for i in range(NCORES)], axis=2
    )
    return out


# revision 11
# speedup vs baseline: 211.7976x; 211.7976x over previous
"""Trainium2 Bass kernel for nn_Cross_Attn (sparse_attention).

Reference computation (B=4, C=384, N=2048, K=16, G=32):
  q  = Wq@feat + bq                            [B,N,C]
  gk = Wk@grouped_feat + bk                    [B,N,C,K]
  s  = (q . gk) * C^-0.5                       [B,N,K]
  p  = softmax_k(mask(s, count))               [B,N,K]   (rows of attn identical)
  v  = relu(GroupNorm_G(Wv@grouped_feat + bv)) [B,C,N,K]
  out[b,c,n] = K * sum_k p[b,n,k] * v[b,c,n,k]

Algebraic restructure used here:
  * attn is rank-1 over the query axis -> out = K * sum_k p * v.
  * s = (Wk^T q) . g + q.bk; the q.bk term is constant over k and softmax
    drops it, so s = u . g with u = (scale Wk^T Wq) feat + scale Wk^T bq.
  * GroupNorm statistics couple all of N, so the kernel runs two SPMD
    launches over N-shards: A computes p + per-channel mean/var (bn_stats
    over v0 = Wv@g); the host merges stats into per-(b,c) affine scale/bias;
    B recomputes v' = (alpha*Wv)@g + bias' and accumulates
    out = sum_k relu(v')*p  (relu(z)*p == relu(z*p) since p>=0).

Sharding: N axis split contiguously across the 8 cores (256 points each);
all weights replicated. Only host-merged 6KB of statistics cross cores.
"""

import numpy as np

import concourse.bass as bass
import concourse.mybir as mybir
import concourse.tile as tile
from concourse import bass_utils

B, C, N, K, G = 4, 384, 2048, 16, 32
EPS = 1e-5
NCORES = 8
NS = N // NCORES          # n-points per core
CT = C // 128             # 128-partition tiles per 384 channels
NHALF = NS // 128         # 128-n' scores tiles per (b, core)
NK = NS * K               # free elems per (b, core)
HNK = 128 * K             # free elems per (b, half)
CH = 512                  # matmul moving chunk (fp32 max, 1 PSUM bank)
NCHUNK = HNK // CH        # 512-col chunks per (b, half)
NPC = CH // K             # n' values covered per chunk (32)
SCALE = float(C) ** -0.5

F32 = mybir.dt.float32
F32R = mybir.dt.float32r
BF16 = mybir.dt.bfloat16

# --- tunables -------------------------------------------------------------
MM_F32R = True                      # big GEMMs as float32r (1 cyc/row vs 4)
B_TT_GPSIMD = False                 # offload half the z*p multiplies to GpSimd
B_P_DOUBLE = False                  # p replication via SBUF->SBUF doubling
DT_MM = F32R if MM_F32R else F32    # dtype of matmul operand tensors/tiles
_wait_counter = [0]


def _fix_excess_waits(nc, max_waits=1):
    """Split instructions carrying more sync waits than this walrus accepts
    (TileContext's tail drain waits on the whole global clock)."""
    for f in nc.m.functions:
        for bb in f.blocks:
            out = []
            for ins in bb.instructions:
                si = ins.sync_info
                if si is not None and si.on_wait and len(si.on_wait) > max_waits:
                    waits = list(si.on_wait)
                    head, tail = waits[:-max_waits], waits[-max_waits:]
                    for i in range(0, len(head), max_waits):
                        _wait_counter[0] += 1
                        nop = mybir.InstNoOp(
                            name=f"I-waitsplit-{_wait_counter[0]}", ins=[], outs=[]
                        )
                        nop.engine = ins.engine
                        nop.sync_info = type(si)(
                            on_wait=head[i : i + max_waits], on_update=[]
                        )
                        out.append(nop)
                    ins.sync_info = type(si)(
                        on_wait=tail, on_update=list(si.on_update or [])
                    )
                out.append(ins)
            bb.instructions[:] = out
    return nc


def build_a(fix=True, reps=1):
    """Launch A: scores+softmax -> p;  bn stats of v0 = Wv@g (no bias)."""
    nc = bass.Bass("TRN2", target_bir_lowering=False, debug=False)
    g_d = nc.dram_tensor("g", [B, C, NS, K], DT_MM, kind="ExternalInput")
    feat_d = nc.dram_tensor("feat", [B, C, NS], DT_MM, kind="ExternalInput")
    count_d = nc.dram_tensor("count", [B, NS], mybir.dt.int32, kind="ExternalInput")
    mt_d = nc.dram_tensor("Mt", [C, C], DT_MM, kind="ExternalInput")
    cvec_d = nc.dram_tensor("cvec", [C], F32, kind="ExternalInput")
    wvt_d = nc.dram_tensor("WvT", [C, C], DT_MM, kind="ExternalInput")
    iota_d = nc.dram_tensor("iota", [128, K], F32, kind="ExternalInput")
    diag_d = nc.dram_tensor("D", [128, 128 * K], F32, kind="ExternalInput")
    p_d = nc.dram_tensor("p", [B, NS, K], F32, kind="ExternalOutput")
    stats_d = nc.dram_tensor("stats", [128, CT, B, 2], F32, kind="ExternalOutput")

    with tile.TileContext(nc) as tc:
        with (
            tc.tile_pool(name="consts", bufs=1) as consts,
            tc.tile_pool(name="gpool", bufs=3) as gpool,
            tc.tile_pool(name="work", bufs=3) as work,
            tc.tile_pool(name="small", bufs=4) as small,
            tc.tile_pool(name="acc", bufs=1) as accp,
            tc.tile_pool(name="ps_u", bufs=2, space="PSUM") as ps_u,
            tc.tile_pool(name="ps_s", bufs=2, space="PSUM") as ps_s,
            tc.tile_pool(name="ps_v", bufs=3, space="PSUM") as ps_v,
        ):
            mt_sb = consts.tile([128, CT, C], DT_MM)
            nc.sync.dma_start(mt_sb[:], mt_d[:].rearrange("(t p) c -> p t c", p=128))
            wvt_sb = consts.tile([128, CT, C], DT_MM)
            nc.sync.dma_start(wvt_sb[:], wvt_d[:].rearrange("(t p) c -> p t c", p=128))
            cvec_sb = consts.tile([128, CT], F32)
            nc.sync.dma_start(cvec_sb[:], cvec_d[:].rearrange("(t p) -> p t", p=128))
            iota_sb = consts.tile([128, K], F32)
            nc.sync.dma_start(iota_sb[:], iota_d[:])
            diag_sb = consts.tile([128, 128 * K], F32)
            nc.sync.dma_start(diag_sb[:], diag_d[:])

            def body():
                # u[b] = Mt^T@feat + cvec, laid out [cu-part, ct, b, n]
                u_sb = accp.tile([128, CT, B, NS], DT_MM, tag="u")
                for b in range(B):
                    feat_t = work.tile([128, CT, NS], DT_MM, tag="feat")
                    nc.sync.dma_start(
                        feat_t[:], feat_d[b].rearrange("(t p) n -> p t n", p=128)
                    )
                    for cu in range(CT):
                        ups = ps_u.tile([128, NS], F32)
                        for cq in range(CT):
                            nc.tensor.matmul(
                                ups[:],
                                mt_sb[:, cq, cu * 128 : (cu + 1) * 128],
                                feat_t[:, cq, :],
                                start=(cq == 0),
                                stop=(cq == CT - 1),
                            )
                        nc.scalar.activation(
                            u_sb[:, cu, b, :],
                            ups[:],
                            mybir.ActivationFunctionType.Identity,
                            bias=cvec_sb[:, cu : cu + 1],
                            scale=1.0,
                        )

                bnrec = accp.tile([128, CT, B, NHALF * NCHUNK, 6], F32, tag="bnrec")
                for b in range(B):
                    for h in range(NHALF):
                        g_sb = gpool.tile([128, CT, HNK], DT_MM, tag="g")
                        for ct in range(CT):
                            nc.sync.dma_start(
                                g_sb[:, ct, :],
                                g_d[b].rearrange("(t p) n k -> p t (n k)", p=128)[
                                    :, ct, h * HNK : (h + 1) * HNK
                                ],
                            )

                        # scores: all-pairs matmul + diagonal extraction
                        sslot = small.tile([128, NCHUNK, K], F32, tag="sslot")
                        for ci in range(NCHUNK):
                            aps = ps_s.tile([128, CH], F32)
                            for ct in range(CT):
                                nc.tensor.matmul(
                                    aps[:],
                                    u_sb[:, ct, b, h * 128 : (h + 1) * 128],
                                    g_sb[:, ct, ci * CH : (ci + 1) * CH],
                                    start=(ct == 0),
                                    stop=(ct == CT - 1),
                                )
                            td = work.tile([128, CH], F32, tag="td")
                            nc.vector.tensor_tensor(
                                td[:],
                                aps[:],
                                diag_sb[:, ci * CH : (ci + 1) * CH],
                                op=mybir.AluOpType.mult,
                            )
                            nc.vector.tensor_reduce(
                                sslot[:, ci, :],
                                td[:].rearrange("p (n k) -> p k n", k=K),
                                axis=mybir.AxisListType.X,
                                op=mybir.AluOpType.add,
                            )
                        s_sb = small.tile([128, K], F32, tag="s")
                        nc.vector.tensor_reduce(
                            s_sb[:],
                            sslot[:].rearrange("p c k -> p k c"),
                            axis=mybir.AxisListType.X,
                            op=mybir.AluOpType.add,
                        )

                        # masked softmax (k<count; count clipped to >=1)
                        cnt_i = small.tile([128, 1], mybir.dt.int32, tag="cnti")
                        nc.sync.dma_start(
                            cnt_i[:],
                            count_d[b, h * 128 : (h + 1) * 128].unsqueeze(-1),
                        )
                        cnt_f = small.tile([128, 1], F32, tag="cntf")
                        nc.vector.tensor_copy(cnt_f[:], cnt_i[:])
                        nc.vector.tensor_scalar_max(cnt_f[:], cnt_f[:], 1.0)
                        m_sb = small.tile([128, K], F32, tag="m")
                        nc.vector.tensor_tensor(
                            m_sb[:],
                            iota_sb[:],
                            cnt_f[:].broadcast_to((128, K)),
                            op=mybir.AluOpType.is_lt,
                        )
                        mx = small.tile([128, 1], F32, tag="mx")
                        nc.vector.tensor_reduce(
                            mx[:], s_sb[:], axis=mybir.AxisListType.X,
                            op=mybir.AluOpType.max,
                        )
                        negmx = small.tile([128, 1], F32, tag="negmx")
                        nc.vector.tensor_scalar_mul(negmx[:], mx[:], -1.0)
                        e_sb = small.tile([128, K], F32, tag="e")
                        nc.scalar.activation(
                            e_sb[:],
                            s_sb[:],
                            mybir.ActivationFunctionType.Exp,
                            bias=negmx[:, 0:1],
                            scale=1.0,
                        )
                        em = small.tile([128, K], F32, tag="em")
                        nc.vector.tensor_tensor(
                            em[:], e_sb[:], m_sb[:], op=mybir.AluOpType.mult
                        )
                        sm = small.tile([128, 1], F32, tag="sm")
                        nc.vector.tensor_reduce(
                            sm[:], em[:], axis=mybir.AxisListType.X,
                            op=mybir.AluOpType.add,
                        )
                        rec = small.tile([128, 1], F32, tag="rec")
                        nc.vector.reciprocal(rec[:], sm[:])
                        nc.vector.tensor_scalar_mul(rec[:], rec[:], float(K))
                        p_t = small.tile([128, K], F32, tag="pt")
                        nc.vector.tensor_scalar_mul(p_t[:], em[:], rec[:, 0:1])
                        nc.sync.dma_start(p_d[b, h * 128 : (h + 1) * 128, :], p_t[:])

                        # v0 = Wv@g -> bn_stats per 512-chunk
                        for co in range(CT):
                            for ci in range(NCHUNK):
                                vps = ps_v.tile([128, CH], F32)
                                for cin in range(CT):
                                    nc.tensor.matmul(
                                        vps[:],
                                        wvt_sb[:, cin, co * 128 : (co + 1) * 128],
                                        g_sb[:, cin, ci * CH : (ci + 1) * CH],
                                        start=(cin == 0),
                                        stop=(cin == CT - 1),
                                    )
                                nc.vector.bn_stats(
                                    bnrec[:, co, b, h * NCHUNK + ci, :], vps[:]
                                )

                stats_sb = accp.tile([128, CT, B, 2], F32, tag="stats")
                for co in range(CT):
                    for b in range(B):
                        nc.vector.bn_aggr(
                            stats_sb[:, co, b, :], bnrec[:, co, b, :, :]
                        )
                nc.sync.dma_start(stats_d[:], stats_sb[:])

            for _ in range(reps):
                body()

    return _fix_excess_waits(nc) if fix else nc


def build_b(fix=True, reps=1):
    """Launch B: out[c,n] = sum_k relu(alpha*(Wv@g) + bias') * p ."""
    nc = bass.Bass("TRN2", target_bir_lowering=False, debug=False)
    g_d = nc.dram_tensor("g", [B, C, NS, K], DT_MM, kind="ExternalInput")
    p_d = nc.dram_tensor("p", [B, NS, K], F32, kind="ExternalInput")
    wvt_d = nc.dram_tensor("WvT", [C, C], DT_MM, kind="ExternalInput")
    sc_d = nc.dram_tensor("scaleB", [C, B], F32, kind="ExternalInput")
    bs_d = nc.dram_tensor("biasB", [C, B], F32, kind="ExternalInput")
    out_d = nc.dram_tensor("out", [B, C, NS], F32, kind="ExternalOutput")

    with tile.TileContext(nc) as tc:
        with (
            tc.tile_pool(name="consts", bufs=1) as consts,
            tc.tile_pool(name="gpool", bufs=3) as gpool,
            tc.tile_pool(name="work", bufs=3) as work,
            tc.tile_pool(name="prep", bufs=2) as prep,
            tc.tile_pool(name="acc", bufs=1) as accp,
            tc.tile_pool(name="ps_v", bufs=4, space="PSUM") as ps_v,
        ):
            wvt_sb = consts.tile([128, CT, C], DT_MM)
            nc.sync.dma_start(wvt_sb[:], wvt_d[:].rearrange("(t p) c -> p t c", p=128))
            # per-(b, cout) affine columns: [p, ct, b]
            sc_sb = consts.tile([128, CT, B], F32)
            nc.sync.dma_start(sc_sb[:], sc_d[:].rearrange("(t p) b -> p t b", p=128))
            bs_sb = consts.tile([128, CT, B], F32)
            nc.sync.dma_start(bs_sb[:], bs_d[:].rearrange("(t p) b -> p t b", p=128))

            def body():
                out_acc = accp.tile([128, CT, B, NS], F32, tag="oacc")
                for b in range(B):
                    # replicate p[b] across partitions
                    p_rep = prep.tile([128, NK], F32, tag="prep")
                    if B_P_DOUBLE:
                        # 64KB HBM read + log2(128) SBUF->SBUF doubling DMAs
                        nc.sync.dma_start(
                            p_rep[0:1, :], p_d[b].rearrange("n k -> () (n k)")
                        )
                        np_done = 1
                        while np_done < 128:
                            cp = min(np_done, 128 - np_done)
                            nc.sync.dma_start(
                                p_rep[np_done : np_done + cp, :], p_rep[0:cp, :]
                            )
                            np_done += cp
                    else:
                        nc.sync.dma_start(
                            p_rep[:],
                            p_d[b].rearrange("n k -> (n k)").unsqueeze(0)
                            .partition_broadcast(128)[:, 0, :],
                        )
                    for h in range(NHALF):
                        g_sb = gpool.tile([128, CT, HNK], DT_MM, tag="g")
                        for ct in range(CT):
                            nc.sync.dma_start(
                                g_sb[:, ct, :],
                                g_d[b].rearrange("(t p) n k -> p t (n k)", p=128)[
                                    :, ct, h * HNK : (h + 1) * HNK
                                ],
                            )
                        for co in range(CT):
                            for ci in range(NCHUNK):
                                vps = ps_v.tile([128, CH], F32)
                                for cin in range(CT):
                                    nc.tensor.matmul(
                                        vps[:],
                                        wvt_sb[:, cin, co * 128 : (co + 1) * 128],
                                        g_sb[:, cin, ci * CH : (ci + 1) * CH],
                                        start=(cin == 0),
                                        stop=(cin == CT - 1),
                                    )
                                # z = relu(alpha*v0 + bias')  (GN affine + relu)
                                z_sb = work.tile([128, CH], F32, tag="z")
                                nc.scalar.activation(
                                    z_sb[:],
                                    vps[:],
                                    mybir.ActivationFunctionType.Relu,
                                    bias=bs_sb[:, co, b : b + 1],
                                    scale=sc_sb[:, co, b : b + 1],
                                )
                                t_sb = work.tile([128, CH], F32, tag="t")
                                mul_eng = (
                                    nc.gpsimd
                                    if (B_TT_GPSIMD and ci % 2 == 1)
                                    else nc.vector
                                )
                                mul_eng.tensor_tensor(
                                    t_sb[:],
                                    z_sb[:],
                                    p_rep[:, h * HNK + ci * CH : h * HNK + (ci + 1) * CH],
                                    op=mybir.AluOpType.mult,
                                )
                                nc.vector.tensor_reduce(
                                    out_acc[
                                        :, co, b,
                                        h * 128 + ci * NPC : h * 128 + (ci + 1) * NPC,
                                    ],
                                    t_sb[:].rearrange("p (n k) -> p n k", k=K),
                                    axis=mybir.AxisListType.X,
                                    op=mybir.AluOpType.add,
                                )
                for co in range(CT):
                    for b in range(B):
                        nc.sync.dma_start(
                            out_d[b, co * 128 : (co + 1) * 128, :],
                            out_acc[:, co, b, :],
                        )

            for _ in range(reps):
                body()

    return _fix_excess_waits(nc) if fix else nc


# ---------------------------------------------------------------------------
_built = {}


def _get_modules():
    if "a" not in _built:
        _built["a"] = build_a()
        _built["b"] = build_b()
    return _built["a"], _built["b"]


def host_prep(Wq, bq, Wk, bk):
    Mt = (SCALE * (Wq.T.astype(np.float64) @ Wk.astype(np.float64))).astype(np.float32)
    cvec = (SCALE * (Wk.T.astype(np.float64) @ bq.astype(np.float64))).astype(
        np.float32
    )
    iota = np.broadcast_to(np.arange(K, dtype=np.float32), (128, K)).copy()
    # D[p, (n,k)] = 1 where the all-pairs column's n matches partition p.
    pidx = np.arange(128)
    nidx = np.arange(128 * K) // K
    D = (pidx[:, None] == nidx[None, :]).astype(np.float32)
    return Mt, cvec, iota, D


def host_stats_to_affine(stats_all, bv, gn_w, gn_b):
    """stats_all: [NCORES, 128, CT, B, 2] -> (scaleB, biasB) each [B, C] f32."""
    st = stats_all.astype(np.float64)
    mean0 = st[..., 0].transpose(2, 1, 0, 3).reshape(C, NCORES, B)
    var0 = st[..., 1].transpose(2, 1, 0, 3).reshape(C, NCORES, B)
    bv64 = bv.astype(np.float64)
    m_c = mean0.mean(axis=1) + bv64[:, None]                         # [C, B]
    e2_c = (var0 + mean0**2).mean(axis=1) + (
        2 * mean0.mean(axis=1) * bv64[:, None] + (bv64**2)[:, None]
    )
    m_g = m_c.reshape(G, C // G, B).mean(axis=1)                     # [G, B]
    e2_g = e2_c.reshape(G, C // G, B).mean(axis=1)
    var_g = e2_g - m_g**2
    rstd = 1.0 / np.sqrt(var_g + EPS)
    rstd_c = np.repeat(rstd, C // G, axis=0)                         # [C, B]
    mu_c = np.repeat(m_g, C // G, axis=0)
    alpha = gn_w.astype(np.float64)[:, None] * rstd_c
    beta = gn_b.astype(np.float64)[:, None] - mu_c * alpha
    scaleB = alpha.T.astype(np.float32)                              # [B, C]
    biasB = (alpha * bv64[:, None] + beta).T.astype(np.float32)
    return scaleB, biasB


def make_in_a(feat, g, count, Wq, bq, Wk, bk, Wv):
    Mt, cvec, iota, D = host_prep(Wq, bq, Wk, bk)
    WvT = np.ascontiguousarray(Wv.T)
    core_sl = [slice(i * NS, (i + 1) * NS) for i in range(NCORES)]
    return [
        {
            "g": g[:, :, sl, :], "feat": feat[:, :, sl], "count": count[:, sl],
            "Mt": Mt, "cvec": cvec, "WvT": WvT, "iota": iota, "D": D,
        }
        for sl in core_sl
    ]


def make_in_b(g, p_all, scaleB, biasB, Wv):
    WvT = np.ascontiguousarray(Wv.T)
    core_sl = [slice(i * NS, (i + 1) * NS) for i in range(NCORES)]
    return [
        {
            "g": g[:, :, core_sl[i], :], "p": p_all[i],
            "WvT": WvT,
            "scaleB": np.ascontiguousarray(scaleB.T),
            "biasB": np.ascontiguousarray(biasB.T),
        }
        for i in range(NCORES)
    ]


def kernel(feat, grouped_feat, count, Wq, bq, Wk, bk, Wv, bv, gn_w, gn_b):
    feat = np.asarray(feat, dtype=np.float32)
    g = np.asarray(grouped_feat, dtype=np.float32)
    count = np.asarray(count, dtype=np.int32)
    Wq, bq, Wk, bk, Wv, bv, gn_w, gn_b = (
        np.asarray(a, dtype=np.float32) for a in (Wq, bq, Wk, bk, Wv, bv, gn_w, gn_b)
    )
    nc_a, nc_b = _get_modules()

    in_a = make_in_a(feat, g, count, Wq, bq, Wk, bk, Wv)
    res_a = bass_utils.run_bass_kernel_spmd(nc_a, in_a, core_ids=list(range(NCORES)))
    stats_all = np.stack([res_a.results[i]["stats"] for i in range(NCORES)])
    p_all = [res_a.results[i]["p"] for i in range(NCORES)]

    scaleB, biasB = host_stats_to_affine(stats_all, bv, gn_w, gn_b)
    in_b = make_in_b(g, p_all, scaleB, biasB, Wv)
    res_b = bass_utils.run_bass_kernel_spmd(nc_b, in_b, core_ids=list(range(NCORES)))
    return np.concatenate([res_b.results[i]["out"] for i in range(NCORES)], axis=2)
